# revision 7
# baseline (speedup 1.0000x reference)
"""Causal multi-head attention (16 heads, hd=64) on 8 trn2 NeuronCores.

Sharding: core c -> batch b = c // 4, head-group g = c % 4 (4 heads = 256
columns of Wq/Wk/Wv).  Each core computes its [S, 256] slice of the three
outputs (attn out, K_cache, V_cache); the host gathers slices.

All-bf16 dataflow (rel-err budget 2e-2; bf16 operand rounding ~4e-3):
  - x, Wq/Wk/Wv, bv arrive bf16 (halves input DMA); out/kct/vc leave bf16
    (halves output DMA); host casts to fp32 in the gather.
  - bf16 matmuls run at the same PE rate as fp32r at N>=256 but (a) avoid
    the 4x penalty on narrow (N<256) diagonal matmuls and (b) get FWL
    (2x faster LDWEIGHTS), which matters for the AV stage below.
  - K/Q projections: KT/QT [c, q] via lhsT = W chunk; per-partition bias
    added during DVE eviction straight to bf16 (ktb doubles as the
    K_cache DMA source -- one eviction serves both).
  - V natural [s, c] (rank-1 bias matmul) -> vc DMA + per-head V_aug
    tiles [k, 65] with a ones column (softmax denominator rides the AV
    matmul for free).
  - scores ST[k, q]: two heads of a pair as concurrent row-tiled matmuls
    (K=64, partition offsets 0/64) into one [128, 1024] psum tile;
    diagonal blocks narrowed; exp on ACT (scale=1/8, per-k pad bias)
    -> bf16 pt; partial triangle zeroed by gpsimd affine_select.
  - AV as out[q, d] = pt^T @ V_aug per 128-query chunk: lhsT = pt chunk
    [128k, 128q] (bf16 FWL), rhs = V_aug [128k, 65], accumulated over
    k-tiles in psum [128, 65].  Output lands directly in [q, d] layout:
    no PE transposes, no transpose evictions.  reciprocal of the ones
    column + tensor_scalar_mul -> bf16 out staging -> DMA.
  - DMA: x in 16 [128, 1024] pieces (half-S-major so early q-slices
    unblock fast), weights per chunk on the gpsimd (SWDGE) queue, vc on
    gpsimd, kct/out on sync.  Fewer+bigger pieces than one-per-tile: the
    ~0.6us per-DMA issue cost was capping input bandwidth.
"""

import numpy as np

P = 128
S = 2048
HIN = 1024
C = 256  # columns per core = 4 heads * 64
HD = 64
NCORES = 8
HC = HIN // P  # 8 contraction chunks
NKT = S // P  # 16 k-tiles
QW = 512  # q-slice width
NQ = S // QW  # 4 q-slices
NPAIR = C // P  # 2 head-pairs per core

_nc_cache = None


def build_nc():
    import concourse.bacc as bacc
    import concourse.mybir as mybir
    from concourse.tile import TileContext
    from contextlib import ExitStack

    f32 = mybir.dt.float32
    bf16 = mybir.dt.bfloat16
    Exp = mybir.ActivationFunctionType.Exp
    is_ge = mybir.AluOpType.is_ge

    nc = bacc.Bacc(None, target_bir_lowering=False)

    xt = nc.declare_dram_parameter("xt", [HIN, S], bf16, isOutput=False)
    wq = nc.declare_dram_parameter("wq", [HIN, C], bf16, isOutput=False)
    wk = nc.declare_dram_parameter("wk", [HIN, C], bf16, isOutput=False)
    wv = nc.declare_dram_parameter("wv", [HIN, C], bf16, isOutput=False)
    bqc = nc.declare_dram_parameter("bqc", [P, NPAIR], f32, isOutput=False)
    bkc = nc.declare_dram_parameter("bkc", [P, NPAIR], f32, isOutput=False)
    bv = nc.declare_dram_parameter("bv", [1, C], bf16, isOutput=False)
    padneg = nc.declare_dram_parameter("padneg", [P, NKT], f32, isOutput=False)
    ones = nc.declare_dram_parameter("ones", [1, P], bf16, isOutput=False)
    out = nc.declare_dram_parameter("out", [S, C], bf16, isOutput=True)
    kct = nc.declare_dram_parameter("kct", [C, S], bf16, isOutput=True)
    vc = nc.declare_dram_parameter("vc", [S, C], bf16, isOutput=True)

    with TileContext(nc) as tc, ExitStack() as ctx:
        persist = ctx.enter_context(tc.tile_pool(name="persist", bufs=1))
        xt_sb = persist.tile([P, HC, S], bf16)
        wq_sb = persist.tile([P, HC, C], bf16)
        wk_sb = persist.tile([P, HC, C], bf16)
        wv_sb = persist.tile([P, HC, C], bf16)
        bqc_sb = persist.tile([P, NPAIR], f32)
        bkc_sb = persist.tile([P, NPAIR], f32)
        bv_sb = persist.tile([1, C], bf16)
        pn_sb = persist.tile([P, NKT], f32)
        ones_sb = persist.tile([1, P], bf16)
        qt_sb = persist.tile([P, NPAIR, S], bf16)
        ktb = persist.tile([P, NPAIR, S], bf16)
        va = persist.tile([P, NKT, 2 * NPAIR, HD + 1], bf16)
        out_sb = persist.tile([P, NKT, C], bf16)
        warm_sb = persist.tile([P, 5 * P], bf16)

        # x pieces ordered by consumption: quarters 0/1 per-chunk (so
        # qi=0/1 projections unblock at the earliest possible moment),
        # back half in [128, 1024] pieces (fewer issue slots); weights
        # per chunk on the SWDGE queue.
        for hh in range(2):
            hsl = slice(hh * QW, (hh + 1) * QW)
            for j in range(HC):
                nc.sync.dma_start(
                    xt_sb[:, j, hsl], xt[j * P : (j + 1) * P, hsl]
                )
        h2 = slice(S // 2, S)
        for j in range(HC):
            nc.sync.dma_start(xt_sb[:, j, h2], xt[j * P : (j + 1) * P, h2])
        nc.gpsimd.dma_start(bqc_sb[:], bqc[:])
        nc.gpsimd.dma_start(bkc_sb[:], bkc[:])
        nc.gpsimd.dma_start(bv_sb[:], bv[:])
        nc.gpsimd.dma_start(pn_sb[:], padneg[:])
        nc.gpsimd.dma_start(ones_sb[:], ones[:])
        for j in range(HC):
            jsl = slice(j * P, (j + 1) * P)
            nc.gpsimd.dma_start(wk_sb[:, j, :], wk[jsl, :])
            nc.gpsimd.dma_start(wq_sb[:, j, :], wq[jsl, :])
            nc.gpsimd.dma_start(wv_sb[:, j, :], wv[jsl, :])
        # ones column of V_aug (position 64); DVE write, not DMA (a
        # strided sub-word DMA write would RMW-race adjacent columns)
        nc.vector.memset(va[:, :, :, HD : HD + 1], 1.0)

        psum = ctx.enter_context(tc.tile_pool(name="psum", bufs=2, space="PSUM"))
        work = ctx.enter_context(tc.tile_pool(name="work", bufs=2))

        # PE warm-up: ~7us of dummy matmuls with no DMA dependency, run
        # while the input DMA streams in.  Trips the HAM activity window
        # so the first real projections execute at 2.4 GHz instead of
        # 1.2 GHz (the PE clock-gate defaults to half rate and needs
        # ~3.4us of sustained activity to lift).
        nc.vector.memset(warm_sb[:], 1.0)
        warm_ps = psum.tile([P, QW], f32, tag="proj", bufs=2, name="warm_ps")
        for _ in range(32):
            nc.tensor.matmul(
                warm_ps, warm_sb[:, :P], warm_sb[:, P:], start=True, stop=True
            )

        def kq_proj(qi):
            qsl = slice(qi * QW, (qi + 1) * QW)
            for p2 in range(NPAIR):
                csl = slice(p2 * P, (p2 + 1) * P)
                for w_sb, b_sb, dst in (
                    (wk_sb, bkc_sb, ktb),
                    (wq_sb, bqc_sb, qt_sb),
                ):
                    ps = psum.tile([P, QW], f32, tag="proj", bufs=2, name="p_ps")
                    for j in range(HC):
                        nc.tensor.matmul(
                            ps, w_sb[:, j, csl], xt_sb[:, j, qsl],
                            start=(j == 0), stop=(j == HC - 1),
                        )
                    nc.vector.tensor_scalar_add(
                        dst[:, p2, qsl], ps, b_sb[:, p2 : p2 + 1]
                    )
                nc.sync.dma_start(
                    kct[p2 * P : (p2 + 1) * P, qsl], ktb[:, p2, qsl]
                )

        def v_wave(qi):
            for i in range(4 * qi, 4 * qi + 4):
                ksl = slice(i * P, (i + 1) * P)
                ps = psum.tile([P, QW], f32, tag="proj", bufs=2, name="v_ps")[:, :C]
                for j in range(HC):
                    nc.tensor.matmul(
                        ps, xt_sb[:, j, ksl], wv_sb[:, j, :],
                        start=(j == 0), stop=False,
                    )
                nc.tensor.matmul(
                    ps, ones_sb[:1, :P], bv_sb[:1, :], start=False, stop=True
                )
                vsb = work.tile([P, C], bf16, tag="vsb", bufs=4, name="vsb")
                nc.vector.tensor_copy(out=vsb[:], in_=ps)
                nc.gpsimd.dma_start(vc[ksl, :], vsb[:])
                for h in range(2 * NPAIR):
                    nc.vector.tensor_copy(
                        out=va[:, i, h, 0:HD], in_=vsb[:, h * HD : (h + 1) * HD]
                    )

        def attention(qi):
            tmax = 4 * qi + 4
            for p2 in range(NPAIR):
                pt = work.tile(
                    [P, NKT, 2, QW], bf16, tag="pt", bufs=2, name="pt"
                )
                for t in range(tmax):
                    ksl = slice(t * P, (t + 1) * P)
                    d = t - 4 * qi
                    W = QW if d < 0 else QW - d * P
                    off = 0 if d < 0 else d * P
                    q0 = qi * QW + off
                    st = psum.tile([P, 2 * QW], f32, tag="st", bufs=2, name="st")
                    nc.tensor.matmul(
                        st[:, 0:W], ktb[0:HD, p2, ksl],
                        qt_sb[0:HD, p2, q0 : q0 + W], start=True, stop=True,
                    )
                    nc.tensor.matmul(
                        st[:, QW : QW + W], ktb[HD:P, p2, ksl],
                        qt_sb[HD:P, p2, q0 : q0 + W], start=True, stop=True,
                    )
                    st3 = st[:].rearrange("p (h w) -> p h w", h=2)[:, :, 0:W]
                    nc.scalar.activation(
                        pt[:, t, :, off : off + W], st3, Exp,
                        bias=pn_sb[:, t : t + 1], scale=0.125,
                    )
                    if d >= 0:
                        nc.gpsimd.affine_select(
                            out=pt[:, t, :, off : off + P],
                            in_=pt[:, t, :, off : off + P],
                            compare_op=is_ge, fill=0.0, base=0,
                            pattern=[[0, 2], [1, P]], channel_multiplier=-1,
                        )
                for qc in range(4):
                    gq = 4 * qi + qc
                    qoff = qc * P
                    for h in range(2):
                        av = psum.tile(
                            [P, QW], f32, tag="av", bufs=2, name="av"
                        )[:, : HD + 1]
                        nt = gq + 1
                        for t in range(nt):
                            nc.tensor.matmul(
                                av, pt[:, t, h, qoff : qoff + P],
                                va[:, t, 2 * p2 + h, :],
                                start=(t == 0), stop=(t == nt - 1),
                            )
                        rcp = work.tile([P, 1], f32, tag="rcp", bufs=4, name="rcp")
                        nc.vector.reciprocal(rcp[:], av[:, HD : HD + 1])
                        col = (2 * p2 + h) * HD
                        nc.vector.tensor_scalar_mul(
                            out_sb[:, gq, col : col + HD], av[:, 0:HD], rcp[:]
                        )
            for qc in range(4):
                gq = 4 * qi + qc
                nc.sync.dma_start(
                    out[gq * P : (gq + 1) * P, :], out_sb[:, gq, :]
                )

        # Projections run one q-slice ahead of attention: proj is
        # DMA-gated early and cheap on PE, while attention's exp work is
        # back-loaded (causal: qi=3 has 4x the exp of qi=0).  Emitting
        # proj(qi+1) at higher priority than attention(qi) finishes all
        # projections early, so the scalar engine (the ~80us exp budget,
        # the co-bottleneck) runs saturated instead of idling early and
        # ping-ponging with the PE at the tail.
        kq_proj(0)
        v_wave(0)
        kq_proj(1)
        v_wave(1)
        attention(0)
        kq_proj(2)
        v_wave(2)
        attention(1)
        kq_proj(3)
        v_wave(3)
        attention(2)
        attention(3)

    nc.finalize()
    return nc


def get_nc():
    global _nc_cache
    if _nc_cache is None:
        _nc_cache = build_nc()
    return _nc_cache


def make_in_maps(x, pad_mask, Wq, bq, Wk, bk, Wv, bv):
    import ml_dtypes

    bf = ml_dtypes.bfloat16
    x = np.asarray(x, np.float32)
    pad_mask = np.asarray(pad_mask, np.float32)
    Wq = np.asarray(Wq, np.float32)
    bq = np.asarray(bq, np.float32)
    Wk = np.asarray(Wk, np.float32)
    bk = np.asarray(bk, np.float32)
    Wv = np.asarray(Wv, np.float32)
    bv = np.asarray(bv, np.float32)
    in_maps = []
    for c in range(NCORES):
        b, g = divmod(c, 4)
        cols = slice(g * C, (g + 1) * C)
        xt = np.ascontiguousarray(x[b].T.astype(bf))  # [HIN, S]
        pn = ((pad_mask[b] - 1.0) * 1e6).reshape(NKT, P).T.copy()  # [P, NKT]
        in_maps.append(
            dict(
                xt=xt,
                ones=np.ones((1, P), bf),
                wq=np.ascontiguousarray(Wq[:, cols].astype(bf)),
                wk=np.ascontiguousarray(Wk[:, cols].astype(bf)),
                wv=np.ascontiguousarray(Wv[:, cols].astype(bf)),
                bqc=np.ascontiguousarray(bq[cols].reshape(NPAIR, P).T),
                bkc=np.ascontiguousarray(bk[cols].reshape(NPAIR, P).T),
                bv=np.ascontiguousarray(bv[cols].reshape(1, C).astype(bf)),
                padneg=pn,
            )
        )
    return in_maps


def gather(results):
    B = 2
    out = np.empty((B, S, HIN), np.float32)
    kcache = np.empty((B, S, HIN), np.float32)
    vcache = np.empty((B, S, HIN), np.float32)
    for c in range(NCORES):
        b, g = divmod(c, 4)
        cols = slice(g * C, (g + 1) * C)
        out[b, :, cols] = results[c]["out"].astype(np.float32)
        kcache[b, :, cols] = results[c]["kct"].T.astype(np.float32)
        vcache[b, :, cols] = results[c]["vc"].astype(np.float32)
    return out, kcache, vcache


def kernel(x, pad_mask, Wq, bq, Wk, bk, Wv, bv):
    from concourse.bass_utils import run_bass_kernel_spmd

    nc = get_nc()
    in_maps = make_in_maps(x, pad_mask, Wq, bq, Wk, bk, Wv, bv)
    res = run_bass_kernel_spmd(nc, in_maps, list(range(NCORES)))
    return gather(res.results)


# revision 8
# speedup vs baseline: 1.1676x; 1.1676x over previous
"""Causal multi-head attention (16 heads, hd=64) on 8 trn2 NeuronCores.

Sharding: core c -> batch b = c // 4, head-group g = c % 4 (4 heads = 256
columns of Wq/Wk/Wv).  Each core computes its [S, 256] slice of the three
outputs (attn out, K_cache, V_cache); the host gathers slices.

Engine streams are STATIC (per-engine execution order == emission order),
so the kernel is emitted as one linear software pipeline clocked by the
exp chain on the scalar engine (the ~75us serial co-bottleneck):

  step (pair, qi, t):  [proj filler piece]  scores(t)  exp(t)  [select]
                       [AV sweeps whose gating exp is >=2 steps old]

  - All-bf16 dataflow: x/W in, out/kct/vc out (host casts); bf16 avoids
    the 4x fp32r penalty on narrow matmuls and gets FWL (2x LDWEIGHTS).
  - Inputs arrive in 6 large DMAs (rearranged dram APs) -- per-piece
    dma_start issue cost (~0.64us each) was rate-limiting delivery.
  - ~14 dummy matmuls with no DMA deps warm the PE clock-gate (HAM)
    during the DMA lead-in so real work runs at 2.4 GHz from the start.
  - K/Q projections: KT/QT [c, q], per-partition bias fused into the DVE
    eviction straight to bf16; ktb doubles as the K_cache DMA source.
  - V natural [s, c] (rank-1 bias matmul) -> vc DMA + per-head V_aug
    [k, 65] tiles with a ones column (denominator rides AV for free).
  - scores: two heads of a pair as concurrent row-tiled matmuls (K=64,
    partition offsets 0/64) into one [128, 1024] psum tile; diagonal
    blocks narrowed; exp on ACT -> bf16 pt; gpsimd affine_select zeroes
    the partial triangle (gpsimd carries nothing else, so selects are
    never queued behind DMA issues).
  - AV sweep (pair, qc, h): out[q, d] += pt[t]^T @ V_aug[t] over t in
    one psum [128, 65] accumulation burst -- emitted only after its
    last exp is 2 steps old, so the burst never stalls mid-group.
    Output lands in [q, d] layout: no PE transposes.  reciprocal of the
    ones column + tensor_scalar_mul -> bf16 out staging -> DMA.
"""

import numpy as np

P = 128
S = 2048
HIN = 1024
C = 256  # columns per core = 4 heads * 64
HD = 64
NCORES = 8
HC = HIN // P  # 8 contraction chunks
NKT = S // P  # 16 k-tiles
QW = 512  # q-slice width
NQ = S // QW  # 4 q-slices
NPAIR = C // P  # 2 head-pairs per core

_nc_cache = None


def build_nc():
    import concourse.bacc as bacc
    import concourse.mybir as mybir
    from concourse.tile import TileContext
    from contextlib import ExitStack
    from collections import deque

    f32 = mybir.dt.float32
    bf16 = mybir.dt.bfloat16
    Exp = mybir.ActivationFunctionType.Exp
    is_ge = mybir.AluOpType.is_ge

    nc = bacc.Bacc(None, target_bir_lowering=False)

    xt = nc.declare_dram_parameter("xt", [HIN, S], bf16, isOutput=False)
    wq = nc.declare_dram_parameter("wq", [HIN, C], bf16, isOutput=False)
    wk = nc.declare_dram_parameter("wk", [HIN, C], bf16, isOutput=False)
    wv = nc.declare_dram_parameter("wv", [HIN, C], bf16, isOutput=False)
    bqc = nc.declare_dram_parameter("bqc", [P, NPAIR], f32, isOutput=False)
    bkc = nc.declare_dram_parameter("bkc", [P, NPAIR], f32, isOutput=False)
    bv = nc.declare_dram_parameter("bv", [1, C], bf16, isOutput=False)
    padneg = nc.declare_dram_parameter("padneg", [P, NKT], f32, isOutput=False)
    ones = nc.declare_dram_parameter("ones", [1, P], bf16, isOutput=False)
    out = nc.declare_dram_parameter("out", [S, C], bf16, isOutput=True)
    kct = nc.declare_dram_parameter("kct", [C, S], bf16, isOutput=True)
    vc = nc.declare_dram_parameter("vc", [S, C], bf16, isOutput=True)

    with TileContext(nc) as tc, ExitStack() as ctx:
        persist = ctx.enter_context(tc.tile_pool(name="persist", bufs=1))
        xt_sb = persist.tile([P, HC, S], bf16)
        wq_sb = persist.tile([P, HC, C], bf16)
        wk_sb = persist.tile([P, HC, C], bf16)
        wv_sb = persist.tile([P, HC, C], bf16)
        bqc_sb = persist.tile([P, NPAIR], f32)
        bkc_sb = persist.tile([P, NPAIR], f32)
        bv_sb = persist.tile([1, C], bf16)
        pn_sb = persist.tile([P, NKT], f32)
        ones_sb = persist.tile([1, P], bf16)
        qt_sb = persist.tile([P, NPAIR, S], bf16)
        ktb = persist.tile([P, NPAIR, S], bf16)
        va = persist.tile([P, NKT, 2 * NPAIR, HD + 1], bf16)
        out_sb = persist.tile([P, NKT, C], bf16)
        vsb = persist.tile([P, NKT, C], bf16)
        warm_sb = persist.tile([P, 5 * P], bf16)

        # inputs in 6 big DMAs on sync (issue cost ~0.64us each; 48
        # per-piece issues would gate delivery at ~200 GB/s): weights
        # whole-tensor via rearranged dram APs, x in quarter 0 /
        # quarter 1 / back-half pieces ordered by consumption.
        nc.sync.dma_start(wk_sb[:], wk[:].rearrange("(j p) c -> p j c", p=P))
        nc.sync.dma_start(wq_sb[:], wq[:].rearrange("(j p) c -> p j c", p=P))
        q0 = slice(0, QW)
        nc.sync.dma_start(
            xt_sb[:, :, q0], xt[:, q0].rearrange("(j p) w -> p j w", p=P)
        )
        nc.sync.dma_start(wv_sb[:], wv[:].rearrange("(j p) c -> p j c", p=P))
        q1 = slice(QW, 2 * QW)
        nc.sync.dma_start(
            xt_sb[:, :, q1], xt[:, q1].rearrange("(j p) w -> p j w", p=P)
        )
        h2 = slice(S // 2, S)
        nc.sync.dma_start(
            xt_sb[:, :, h2], xt[:, h2].rearrange("(j p) w -> p j w", p=P)
        )
        # consts on the scalar queue (idle until the first exp)
        nc.scalar.dma_start(bqc_sb[:], bqc[:])
        nc.scalar.dma_start(bkc_sb[:], bkc[:])
        nc.scalar.dma_start(bv_sb[:], bv[:])
        nc.scalar.dma_start(pn_sb[:], padneg[:])
        nc.scalar.dma_start(ones_sb[:], ones[:])
        # ones column of V_aug (position 64); DVE write, not DMA (a
        # strided sub-word DMA write would RMW-race adjacent columns)
        nc.vector.memset(va[:, :, :, HD : HD + 1], 1.0)

        psum = ctx.enter_context(tc.tile_pool(name="psum", bufs=2, space="PSUM"))
        work = ctx.enter_context(tc.tile_pool(name="work", bufs=2))

        # PE warm-up: ~3us of dummy matmuls with no DMA dependency, run
        # while the input DMA streams in, to trip the HAM activity
        # window (PE clock defaults to 1.2 GHz; needs ~3.4us of
        # sustained activity to reach 2.4 GHz).
        nc.vector.memset(warm_sb[:], 1.0)
        warm_ps = psum.tile([P, QW], f32, tag="proj", bufs=2, name="warm_ps")
        for _ in range(14):
            nc.tensor.matmul(
                warm_ps, warm_sb[:, :P], warm_sb[:, P:], start=True, stop=True
            )

        # ---- emission pieces -------------------------------------------
        def kq_piece(qi, p2, which):
            qsl = slice(qi * QW, (qi + 1) * QW)
            csl = slice(p2 * P, (p2 + 1) * P)
            w_sb, b_sb, dst = (
                (wk_sb, bkc_sb, ktb) if which == "k" else (wq_sb, bqc_sb, qt_sb)
            )
            ps = psum.tile([P, QW], f32, tag="proj", bufs=2, name="p_ps")
            for j in range(HC):
                nc.tensor.matmul(
                    ps, w_sb[:, j, csl], xt_sb[:, j, qsl],
                    start=(j == 0), stop=(j == HC - 1),
                )
            nc.vector.tensor_scalar_add(dst[:, p2, qsl], ps, b_sb[:, p2 : p2 + 1])
            if which == "k":
                nc.sync.dma_start(kct[p2 * P : (p2 + 1) * P, qsl], ktb[:, p2, qsl])

        def v_piece(i):
            ksl = slice(i * P, (i + 1) * P)
            ps = psum.tile([P, QW], f32, tag="proj", bufs=2, name="v_ps")[:, :C]
            for j in range(HC):
                nc.tensor.matmul(
                    ps, xt_sb[:, j, ksl], wv_sb[:, j, :],
                    start=(j == 0), stop=False,
                )
            nc.tensor.matmul(
                ps, ones_sb[:1, :P], bv_sb[:1, :], start=False, stop=True
            )
            nc.vector.tensor_copy(out=vsb[:, i, :], in_=ps)
            nc.sync.dma_start(vc[ksl, :], vsb[:, i, :])
            for h in range(2 * NPAIR):
                nc.vector.tensor_copy(
                    out=va[:, i, h, 0:HD],
                    in_=vsb[:, i, h * HD : (h + 1) * HD],
                )

        def scores_exp(qi, p2, t, pt):
            ksl = slice(t * P, (t + 1) * P)
            d = t - 4 * qi
            W = QW if d < 0 else QW - d * P
            off = 0 if d < 0 else d * P
            qg = qi * QW + off
            st = psum.tile([P, 2 * QW], f32, tag="st", bufs=2, name="st")
            nc.tensor.matmul(
                st[:, 0:W], ktb[0:HD, p2, ksl],
                qt_sb[0:HD, p2, qg : qg + W], start=True, stop=True,
            )
            nc.tensor.matmul(
                st[:, QW : QW + W], ktb[HD:P, p2, ksl],
                qt_sb[HD:P, p2, qg : qg + W], start=True, stop=True,
            )
            st3 = st[:].rearrange("p (h w) -> p h w", h=2)[:, :, 0:W]
            nc.scalar.activation(
                pt[:, t, :, off : off + W], st3, Exp,
                bias=pn_sb[:, t : t + 1], scale=0.125,
            )
            if d >= 0:
                nc.gpsimd.affine_select(
                    out=pt[:, t, :, off : off + P],
                    in_=pt[:, t, :, off : off + P],
                    compare_op=is_ge, fill=0.0, base=0,
                    pattern=[[0, 2], [1, P]], channel_multiplier=-1,
                )

        outcnt = {}

        def sweep(qi, p2, qc, pt):
            gq = 4 * qi + qc
            qoff = qc * P
            nt = gq + 1
            for h in range(2):
                av = psum.tile([P, QW], f32, tag="av", bufs=2, name="av")[
                    :, : HD + 1
                ]
                for t in range(nt):
                    nc.tensor.matmul(
                        av, pt[:, t, h, qoff : qoff + P],
                        va[:, t, 2 * p2 + h, :],
                        start=(t == 0), stop=(t == nt - 1),
                    )
                rcp = work.tile([P, 1], f32, tag="rcp", bufs=4, name="rcp")
                nc.vector.reciprocal(rcp[:], av[:, HD : HD + 1])
                col = (2 * p2 + h) * HD
                nc.vector.tensor_scalar_mul(
                    out_sb[:, gq, col : col + HD], av[:, 0:HD], rcp[:]
                )
            outcnt[gq] = outcnt.get(gq, 0) + 1
            if outcnt[gq] == NPAIR:
                nc.sync.dma_start(out[gq * P : (gq + 1) * P, :], out_sb[:, gq, :])

        # ---- the linear software pipeline ------------------------------
        # first pair's projections ahead of the pipeline; everything else
        # is paced as filler between exp steps
        kq_piece(0, 0, "k")
        kq_piece(0, 0, "q")

        projq = deque()
        projq.append(lambda: kq_piece(0, 1, "k"))
        projq.append(lambda: kq_piece(0, 1, "q"))
        for i in range(4):
            projq.append(lambda i=i: v_piece(i))
        for qi in range(1, NQ):
            projq.append(lambda qi=qi: kq_piece(qi, 0, "k"))
            projq.append(lambda qi=qi: kq_piece(qi, 0, "q"))
            projq.append(lambda qi=qi: kq_piece(qi, 1, "k"))
            projq.append(lambda qi=qi: kq_piece(qi, 1, "q"))
            for i in range(4 * qi, 4 * qi + 4):
                projq.append(lambda i=i: v_piece(i))

        pend = deque()  # (gate_step, qi, p2, qc, pt)
        step = 0
        LAG = 2
        pt_tiles = {}
        for qi in range(NQ):
            for p2 in range(NPAIR):
                pt = work.tile([P, NKT, 2, QW], bf16, tag="pt", bufs=2, name="pt")
                pt_tiles[(qi, p2)] = pt
                for t in range(4 * qi + 4):
                    if projq:
                        projq.popleft()()
                    scores_exp(qi, p2, t, pt)
                    d = t - 4 * qi
                    if d >= 0:
                        pend.append((step, qi, p2, d, pt))
                    while pend and pend[0][0] <= step - LAG:
                        _, sqi, sp2, sqc, spt = pend.popleft()
                        sweep(sqi, sp2, sqc, spt)
                    step += 1
        while projq:
            projq.popleft()()
        while pend:
            _, sqi, sp2, sqc, spt = pend.popleft()
            sweep(sqi, sp2, sqc, spt)

    nc.finalize()
    return nc


def get_nc():
    global _nc_cache
    if _nc_cache is None:
        _nc_cache = build_nc()
    return _nc_cache


def make_in_maps(x, pad_mask, Wq, bq, Wk, bk, Wv, bv):
    import ml_dtypes

    bf = ml_dtypes.bfloat16
    x = np.asarray(x, np.float32)
    pad_mask = np.asarray(pad_mask, np.float32)
    Wq = np.asarray(Wq, np.float32)
    bq = np.asarray(bq, np.float32)
    Wk = np.asarray(Wk, np.float32)
    bk = np.asarray(bk, np.float32)
    Wv = np.asarray(Wv, np.float32)
    bv = np.asarray(bv, np.float32)
    in_maps = []
    for c in range(NCORES):
        b, g = divmod(c, 4)
        cols = slice(g * C, (g + 1) * C)
        xt = np.ascontiguousarray(x[b].T.astype(bf))  # [HIN, S]
        pn = ((pad_mask[b] - 1.0) * 1e6).reshape(NKT, P).T.copy()  # [P, NKT]
        in_maps.append(
            dict(
                xt=xt,
                ones=np.ones((1, P), bf),
                wq=np.ascontiguousarray(Wq[:, cols].astype(bf)),
                wk=np.ascontiguousarray(Wk[:, cols].astype(bf)),
                wv=np.ascontiguousarray(Wv[:, cols].astype(bf)),
                bqc=np.ascontiguousarray(bq[cols].reshape(NPAIR, P).T),
                bkc=np.ascontiguousarray(bk[cols].reshape(NPAIR, P).T),
                bv=np.ascontiguousarray(bv[cols].reshape(1, C).astype(bf)),
                padneg=pn,
            )
        )
    return in_maps


def gather(results):
    B = 2
    out = np.empty((B, S, HIN), np.float32)
    kcache = np.empty((B, S, HIN), np.float32)
    vcache = np.empty((B, S, HIN), np.float32)
    for c in range(NCORES):
        b, g = divmod(c, 4)
        cols = slice(g * C, (g + 1) * C)
        out[b, :, cols] = results[c]["out"].astype(np.float32)
        kcache[b, :, cols] = results[c]["kct"].T.astype(np.float32)
        vcache[b, :, cols] = results[c]["vc"].astype(np.float32)
    return out, kcache, vcache


def kernel(x, pad_mask, Wq, bq, Wk, bk, Wv, bv):
    from concourse.bass_utils import run_bass_kernel_spmd

    nc = get_nc()
    in_maps = make_in_maps(x, pad_mask, Wq, bq, Wk, bk, Wv, bv)
    res = run_bass_kernel_spmd(nc, in_maps, list(range(NCORES)))
    return gather(res.results)


# revision 12
# speedup vs baseline: 1.5120x; 1.2950x over previous
"""Causal multi-head attention (16 heads, hd=64) on 8 trn2 NeuronCores.

Sharding: core c -> batch b = c // 4, head-group g = c % 4 (4 heads = 256
columns of Wq/Wk/Wv).  Each core computes its [S, 256] slice of the three
outputs (attn out, K_cache, V_cache); the host gathers slices.

Engine streams are STATIC (per-engine execution order == emission order),
so the kernel is emitted as one linear software pipeline clocked by the
exp chain on the scalar engine (the ~75us serial co-bottleneck):

  step (pair, qi, t):  [proj filler piece]  scores(t)  exp(t)  [select]
                       [AV sweeps whose gating exp is >=2 steps old]

  - All-bf16 dataflow: x/W in, out/kct/vc out (host casts); bf16 avoids
    the 4x fp32r penalty on narrow matmuls and gets FWL (2x LDWEIGHTS).
  - Inputs arrive in 6 large DMAs (rearranged dram APs) -- per-piece
    dma_start issue cost (~0.64us each) was rate-limiting delivery.
  - ~14 dummy matmuls with no DMA deps warm the PE clock-gate (HAM)
    during the DMA lead-in so real work runs at 2.4 GHz from the start.
  - K/Q projections: KT/QT [c, q], per-partition bias fused into the DVE
    eviction straight to bf16; ktb doubles as the K_cache DMA source.
  - V natural [s, c] (rank-1 bias matmul) -> vc DMA + per-head V_aug
    [k, 65] tiles with a ones column (denominator rides AV for free).
  - scores: two heads of a pair as concurrent row-tiled matmuls (K=64,
    partition offsets 0/64) into one [128, 1024] psum tile; diagonal
    blocks narrowed; exp on ACT -> bf16 pt; gpsimd affine_select zeroes
    the partial triangle (gpsimd carries nothing else, so selects are
    never queued behind DMA issues).
  - AV sweep (pair, qc, h): out[q, d] += pt[t]^T @ V_aug[t] over t in
    one psum [128, 65] accumulation burst -- emitted only after its
    last exp is 2 steps old, so the burst never stalls mid-group.
    Output lands in [q, d] layout: no PE transposes.  reciprocal of the
    ones column + tensor_scalar_mul -> bf16 out staging -> DMA.
"""

import numpy as np

P = 128
S = 2048
HIN = 1024
C = 256  # columns per core = 4 heads * 64
HD = 64
NCORES = 8
HC = HIN // P  # 8 contraction chunks
NKT = S // P  # 16 k-tiles
QW = 512  # q-slice width
NQ = S // QW  # 4 q-slices
NPAIR = C // P  # 2 head-pairs per core

_nc_cache = None


def build_nc():
    import concourse.bacc as bacc
    import concourse.mybir as mybir
    from concourse.tile import TileContext
    from contextlib import ExitStack
    from collections import deque

    f32 = mybir.dt.float32
    bf16 = mybir.dt.bfloat16
    Exp = mybir.ActivationFunctionType.Exp
    is_ge = mybir.AluOpType.is_ge

    nc = bacc.Bacc(None, target_bir_lowering=False)

    xt = nc.declare_dram_parameter("xt", [HIN, S], bf16, isOutput=False)
    wq = nc.declare_dram_parameter("wq", [HIN, C], bf16, isOutput=False)
    wk = nc.declare_dram_parameter("wk", [HIN, C], bf16, isOutput=False)
    wv = nc.declare_dram_parameter("wv", [HIN, C], bf16, isOutput=False)
    bqc = nc.declare_dram_parameter("bqc", [P, NPAIR], f32, isOutput=False)
    bkc = nc.declare_dram_parameter("bkc", [P, NPAIR], f32, isOutput=False)
    bv = nc.declare_dram_parameter("bv", [1, C], bf16, isOutput=False)
    padneg = nc.declare_dram_parameter("padneg", [P, NKT], f32, isOutput=False)
    ones = nc.declare_dram_parameter("ones", [1, P], bf16, isOutput=False)
    out = nc.declare_dram_parameter("out", [S, C], bf16, isOutput=True)
    kct = nc.declare_dram_parameter("kct", [C, S], bf16, isOutput=True)
    vc = nc.declare_dram_parameter("vc", [S, C], bf16, isOutput=True)

    with TileContext(nc) as tc, ExitStack() as ctx:
        persist = ctx.enter_context(tc.tile_pool(name="persist", bufs=1))
        xt_sb = persist.tile([P, HC, S], bf16)
        wq_sb = persist.tile([P, HC, C], bf16)
        wk_sb = persist.tile([P, HC, C], bf16)
        wv_sb = persist.tile([P, HC, C], bf16)
        bqc_sb = persist.tile([P, NPAIR], f32)
        bkc_sb = persist.tile([P, NPAIR], f32)
        bv_sb = persist.tile([1, C], bf16)
        pn_sb = persist.tile([P, NKT], f32)
        ones_sb = persist.tile([1, P], bf16)
        qt_sb = persist.tile([P, NPAIR, S], bf16)
        ktb = persist.tile([P, NPAIR, S], bf16)
        va = persist.tile([P, NKT, 2 * NPAIR, HD + 1], bf16)
        out_sb = persist.tile([P, NKT, C], bf16)
        vsb = persist.tile([P, NKT, C], bf16)

        # inputs in 6 big DMAs on sync (issue cost ~0.64us each; 48
        # per-piece issues would gate delivery at ~200 GB/s): weights
        # whole-tensor via rearranged dram APs, x in quarter 0 /
        # quarter 1 / back-half pieces ordered by consumption.
        q0 = slice(0, QW)
        nc.sync.dma_start(
            xt_sb[:, :, q0], xt[:, q0].rearrange("(j p) w -> p j w", p=P)
        )
        nc.sync.dma_start(wk_sb[:], wk[:].rearrange("(j p) c -> p j c", p=P))
        nc.sync.dma_start(wq_sb[:], wq[:].rearrange("(j p) c -> p j c", p=P))
        nc.sync.dma_start(wv_sb[:], wv[:].rearrange("(j p) c -> p j c", p=P))
        q1 = slice(QW, 2 * QW)
        nc.sync.dma_start(
            xt_sb[:, :, q1], xt[:, q1].rearrange("(j p) w -> p j w", p=P)
        )
        h2 = slice(S // 2, S)
        nc.sync.dma_start(
            xt_sb[:, :, h2], xt[:, h2].rearrange("(j p) w -> p j w", p=P)
        )
        # consts on the scalar queue (idle until the first exp)
        nc.scalar.dma_start(bqc_sb[:], bqc[:])
        nc.scalar.dma_start(bkc_sb[:], bkc[:])
        nc.scalar.dma_start(bv_sb[:], bv[:])
        nc.scalar.dma_start(pn_sb[:], padneg[:])
        nc.scalar.dma_start(ones_sb[:], ones[:])
        # ones column of V_aug (position 64); DVE write, not DMA (a
        # strided sub-word DMA write would RMW-race adjacent columns)
        nc.vector.memset(va[:, :, :, HD : HD + 1], 1.0)

        psum = ctx.enter_context(tc.tile_pool(name="psum", bufs=2, space="PSUM"))
        work = ctx.enter_context(tc.tile_pool(name="work", bufs=2))

        # preload the exp table set (~2.7us) during the x-DMA lead-in so
        # the first real exp doesn't pay it on the critical path
        tblw = work.tile([1, 1], f32, tag="tblw", bufs=1, name="tblw")
        nc.scalar.activation(
            tblw[:], pn_sb[:1, 0:1], Exp, bias=pn_sb[:1, 0:1], scale=0.0
        )

        # ---- emission pieces -------------------------------------------
        def kq_piece(qi, p2, which):
            qsl = slice(qi * QW, (qi + 1) * QW)
            csl = slice(p2 * P, (p2 + 1) * P)
            w_sb, b_sb, dst = (
                (wk_sb, bkc_sb, ktb) if which == "k" else (wq_sb, bqc_sb, qt_sb)
            )
            ps = psum.tile([P, QW], f32, tag="proj", bufs=2, name="p_ps")
            for j in range(HC):
                nc.tensor.matmul(
                    ps, w_sb[:, j, csl], xt_sb[:, j, qsl],
                    start=(j == 0), stop=(j == HC - 1),
                )
            nc.vector.tensor_scalar_add(dst[:, p2, qsl], ps, b_sb[:, p2 : p2 + 1])
            if which == "k":
                nc.sync.dma_start(kct[p2 * P : (p2 + 1) * P, qsl], ktb[:, p2, qsl])

        def v_piece(i):
            ksl = slice(i * P, (i + 1) * P)
            ps = psum.tile([P, QW], f32, tag="proj", bufs=2, name="v_ps")[:, :C]
            for j in range(HC):
                nc.tensor.matmul(
                    ps, xt_sb[:, j, ksl], wv_sb[:, j, :],
                    start=(j == 0), stop=False,
                )
            nc.tensor.matmul(
                ps, ones_sb[:1, :P], bv_sb[:1, :], start=False, stop=True
            )
            nc.vector.tensor_copy(out=vsb[:, i, :], in_=ps)
            nc.sync.dma_start(vc[ksl, :], vsb[:, i, :])
            for h in range(2 * NPAIR):
                nc.vector.tensor_copy(
                    out=va[:, i, h, 0:HD],
                    in_=vsb[:, i, h * HD : (h + 1) * HD],
                )

        def scores_exp(qi, p2, t, pt):
            ksl = slice(t * P, (t + 1) * P)
            d = t - 4 * qi
            W = QW if d < 0 else QW - d * P
            off = 0 if d < 0 else d * P
            qg = qi * QW + off
            st = psum.tile([P, 2 * QW], f32, tag="st", bufs=2, name="st")
            nc.tensor.matmul(
                st[:, 0:W], ktb[0:HD, p2, ksl],
                qt_sb[0:HD, p2, qg : qg + W], start=True, stop=True,
            )
            nc.tensor.matmul(
                st[:, QW : QW + W], ktb[HD:P, p2, ksl],
                qt_sb[HD:P, p2, qg : qg + W], start=True, stop=True,
            )
            st3 = st[:].rearrange("p (h w) -> p h w", h=2)[:, :, 0:W]
            nc.scalar.activation(
                pt[:, t, :, off : off + W], st3, Exp,
                bias=pn_sb[:, t : t + 1], scale=0.125,
            )
            if d >= 0:
                nc.gpsimd.affine_select(
                    out=pt[:, t, :, off : off + P],
                    in_=pt[:, t, :, off : off + P],
                    compare_op=is_ge, fill=0.0, base=0,
                    pattern=[[0, 2], [1, P]], channel_multiplier=-1,
                )

        outcnt = {}

        def sweep(qi, p2, qc, pt):
            gq = 4 * qi + qc
            qoff = qc * P
            nt = gq + 1
            for h in range(2):
                av = psum.tile([P, QW], f32, tag="av", bufs=2, name="av")[
                    :, : HD + 1
                ]
                for t in range(nt):
                    nc.tensor.matmul(
                        av, pt[:, t, h, qoff : qoff + P],
                        va[:, t, 2 * p2 + h, :],
                        start=(t == 0), stop=(t == nt - 1),
                    )
                rcp = work.tile([P, 1], f32, tag="rcp", bufs=4, name="rcp")
                nc.vector.reciprocal(rcp[:], av[:, HD : HD + 1])
                col = (2 * p2 + h) * HD
                nc.vector.tensor_scalar_mul(
                    out_sb[:, gq, col : col + HD], av[:, 0:HD], rcp[:]
                )
            outcnt[gq] = outcnt.get(gq, 0) + 1
            if outcnt[gq] == NPAIR:
                nc.sync.dma_start(out[gq * P : (gq + 1) * P, :], out_sb[:, gq, :])

        # ---- the linear software pipeline ------------------------------
        # first pair's projections ahead of the pipeline; everything else
        # is paced as filler on a deadline-driven schedule (emitting one
        # piece every step starves the exp chain -- ~1.3us of PE work per
        # ~1us exp step)
        kq_piece(0, 0, "k")
        kq_piece(0, 0, "q")

        pieces = {
            0: lambda: kq_piece(0, 1, "k"),
            1: lambda: kq_piece(0, 1, "q"),
            2: lambda: v_piece(0),
            3: lambda: v_piece(1),
            4: lambda: v_piece(2),
            5: lambda: v_piece(3),
            6: lambda: kq_piece(1, 0, "k"),
            7: lambda: kq_piece(1, 0, "q"),
            10: lambda: kq_piece(1, 1, "k"),
            11: lambda: kq_piece(1, 1, "q"),
            12: lambda: v_piece(4),
            13: lambda: v_piece(5),
            14: lambda: v_piece(6),
            15: lambda: v_piece(7),
            18: lambda: kq_piece(2, 0, "k"),
            20: lambda: kq_piece(2, 0, "q"),
            26: lambda: kq_piece(2, 1, "k"),
            28: lambda: kq_piece(2, 1, "q"),
            29: lambda: v_piece(8),
            30: lambda: v_piece(9),
            31: lambda: v_piece(10),
            32: lambda: v_piece(11),
            40: lambda: kq_piece(3, 0, "k"),
            43: lambda: kq_piece(3, 0, "q"),
            56: lambda: kq_piece(3, 1, "k"),
            58: lambda: kq_piece(3, 1, "q"),
            59: lambda: v_piece(12),
            60: lambda: v_piece(13),
            61: lambda: v_piece(14),
            62: lambda: v_piece(15),
        }

        pend = deque()  # (gate_step, qi, p2, qc, pt)
        step = 0
        LAG = 2
        for qi in range(NQ):
            for p2 in range(NPAIR):
                pt = work.tile([P, NKT, 2, QW], bf16, tag="pt", bufs=2, name="pt")
                for t in range(4 * qi + 4):
                    if step in pieces:
                        pieces.pop(step)()
                    scores_exp(qi, p2, t, pt)
                    d = t - 4 * qi
                    if d >= 0:
                        pend.append((step, qi, p2, d, pt))
                    while pend and pend[0][0] <= step - LAG:
                        _, sqi, sp2, sqc, spt = pend.popleft()
                        sweep(sqi, sp2, sqc, spt)
                    step += 1
        while pend:
            _, sqi, sp2, sqc, spt = pend.popleft()
            sweep(sqi, sp2, sqc, spt)

    nc.finalize()
    return nc


def get_nc():
    global _nc_cache
    if _nc_cache is None:
        _nc_cache = build_nc()
    return _nc_cache


def make_in_maps(x, pad_mask, Wq, bq, Wk, bk, Wv, bv):
    import ml_dtypes

    bf = ml_dtypes.bfloat16
    x = np.asarray(x, np.float32)
    pad_mask = np.asarray(pad_mask, np.float32)
    Wq = np.asarray(Wq, np.float32)
    bq = np.asarray(bq, np.float32)
    Wk = np.asarray(Wk, np.float32)
    bk = np.asarray(bk, np.float32)
    Wv = np.asarray(Wv, np.float32)
    bv = np.asarray(bv, np.float32)
    in_maps = []
    for c in range(NCORES):
        b, g = divmod(c, 4)
        cols = slice(g * C, (g + 1) * C)
        xt = np.ascontiguousarray(x[b].T.astype(bf))  # [HIN, S]
        pn = ((pad_mask[b] - 1.0) * 1e6).reshape(NKT, P).T.copy()  # [P, NKT]
        in_maps.append(
            dict(
                xt=xt,
                ones=np.ones((1, P), bf),
                wq=np.ascontiguousarray(Wq[:, cols].astype(bf)),
                wk=np.ascontiguousarray(Wk[:, cols].astype(bf)),
                wv=np.ascontiguousarray(Wv[:, cols].astype(bf)),
                bqc=np.ascontiguousarray(bq[cols].reshape(NPAIR, P).T),
                bkc=np.ascontiguousarray(bk[cols].reshape(NPAIR, P).T),
                bv=np.ascontiguousarray(bv[cols].reshape(1, C).astype(bf)),
                padneg=pn,
            )
        )
    return in_maps


def gather(results):
    B = 2
    out = np.empty((B, S, HIN), np.float32)
    kcache = np.empty((B, S, HIN), np.float32)
    vcache = np.empty((B, S, HIN), np.float32)
    for c in range(NCORES):
        b, g = divmod(c, 4)
        cols = slice(g * C, (g + 1) * C)
        out[b, :, cols] = results[c]["out"].astype(np.float32)
        kcache[b, :, cols] = results[c]["kct"].T.astype(np.float32)
        vcache[b, :, cols] = results[c]["vc"].astype(np.float32)
    return out, kcache, vcache


def kernel(x, pad_mask, Wq, bq, Wk, bk, Wv, bv):
    from concourse.bass_utils import run_bass_kernel_spmd

    nc = get_nc()
    in_maps = make_in_maps(x, pad_mask, Wq, bq, Wk, bk, Wv, bv)
    res = run_bass_kernel_spmd(nc, in_maps, list(range(NCORES)))
    return gather(res.results)
